# revision 15
# baseline (speedup 1.0000x reference)
"""Trainium2 Bass kernel for nn_AR_14328010899741.

The reference module runs a linear autoregressive scan: starting from the
rolling window buf0 = y.transpose(0,2,1)[:, :, -168:], each of 24 horizon
steps computes pred = buf @ w + b and shifts it into the buffer. Because
every step is linear, the whole scan collapses to

    out[b, h, c] = sum_n A[h, n] * y[b, n, c] + beta[h] * b_scalar

with A [24, 168] / beta [24] computed on the host from (w, b) by running
the same recurrence on basis vectors (float64, ~700k flops). x is unused.

On device this is a memory-bound batched matmul. Sharding: data-parallel
over batch, 32 batches per core across 8 cores. Per core we stream
y_shard [32, 168, 1024] through SBUF in 2-batch chunks, contract the
T=168 dim on the PE (K split 128+40, fp32r at full PE rate, fp32 PSUM
accumulate), add the bias on the DVE during the PSUM->SBUF copy, and
stream out [32, 24, 1024].
"""

import sys

for _p in ("/opt/trn_rl_repo", "/root/.axon_site", "/root/.axon_site/_ro/trn_rl_repo"):
    if _p not in sys.path:
        sys.path.append(_p)

import numpy as np

B, T, C = 256, 168, 1024
N_SEQ = 168
HORIZON = 24
N_CORES = 8
BPC = B // N_CORES          # batches per core
PAIR = 4                    # batches per DMA iteration
K1 = 128                    # first contraction chunk
K2 = N_SEQ - K1             # second contraction chunk (40)
NCHUNK = 512                # matmul moving free dim / PSUM bank

_RUNNER = None


def _round_fp32r(a: np.ndarray) -> np.ndarray:
    """Round fp32 to the fp32r grid (11 mantissa bits, RNE) — matches
    neuronxcc static_cast_fp32_to_fp32r bit-exactly."""
    u = np.ascontiguousarray(a, dtype=np.float32).view(np.uint32)
    r = (u + np.uint32(0x7FF) + ((u >> np.uint32(12)) & np.uint32(1))) & np.uint32(
        0xFFFFF000
    )
    return r.view(np.float32)


def _coeffs(w: np.ndarray, b: np.ndarray):
    """Unroll the AR scan into A [H, N_SEQ] and bias vector [H] (float64)."""
    wv = w[0].astype(np.float64)
    bv = np.float64(b[0])
    coef = np.eye(N_SEQ, dtype=np.float64)      # buffer coeffs wrt initial window
    const = np.zeros(N_SEQ, dtype=np.float64)   # buffer coeffs wrt the bias b
    A = np.zeros((HORIZON, N_SEQ), dtype=np.float64)
    beta = np.zeros(HORIZON, dtype=np.float64)
    for t in range(HORIZON):
        a = wv @ coef
        c = wv @ const + 1.0
        A[t] = a
        beta[t] = c
        coef = np.vstack([coef[1:], a])
        const = np.concatenate([const[1:], [c]])
    return A.astype(np.float32), (beta * bv).astype(np.float32)


def _build():
    import concourse.bass as bass
    import concourse.bacc as bacc
    import concourse.mybir as mybir
    import concourse.tile as tile
    from concourse.bass_utils import run_bass_kernel_spmd

    f32 = mybir.dt.float32
    f32r = mybir.dt.float32r
    Identity = mybir.ActivationFunctionType.Identity

    # Bacc (not raw Bass): its generate_event_semaphores pass splits
    # multi-semaphore waits into EventSemaphore instructions, which the
    # single-wait-slot HW instructions (notably the fused fp32r matmul)
    # require.
    nc = bacc.Bacc("TRN2", target_bir_lowering=False)
    y_d = nc.dram_tensor("y", [BPC, T, C], f32r, kind="ExternalInput")
    a1_d = nc.dram_tensor("a1", [K1, HORIZON], f32r, kind="ExternalInput")
    a2_d = nc.dram_tensor("a2", [K2, HORIZON], f32r, kind="ExternalInput")
    bias_d = nc.dram_tensor("bias", [HORIZON, 1], f32, kind="ExternalInput")
    out_d = nc.dram_tensor("out", [BPC, HORIZON, C], f32, kind="ExternalOutput")

    # The 40 tail rows (t2) live on SBUF partitions 64..103: DMA engine load
    # balance — partitions 0-63 are served by the even SDMA engines and
    # 64-127 by the odd ones, so the double-loaded tail partitions and the
    # 24-partition output stores land on mostly disjoint engine sets.
    P2 = 64

    with tile.TileContext(nc) as tc:
        with (
            tc.tile_pool(name="consts", bufs=1) as consts,
            tc.tile_pool(name="load1", bufs=3) as load1,
            tc.tile_pool(name="load2", bufs=2) as load2,
            tc.tile_pool(name="store", bufs=4) as store,
            tc.tile_pool(name="psum", bufs=7, space="PSUM") as psum,
            tc.tile_pool(name="wps", bufs=1, space="PSUM") as wps,
        ):
            a1 = consts.tile([K1, HORIZON], f32r)
            a2f = consts.tile([P2 + K2, HORIZON], f32r)
            a2 = a2f[P2 : P2 + K2]
            bias = consts.tile([HORIZON, 1], f32)
            nc.scalar.dma_start(a1[:], a1_d[:])
            nc.scalar.dma_start(a2, a2_d[:])
            nc.scalar.dma_start(bias[:], bias_d[:])

            # fp32r matmuls do not register as PE activity for the HAM clock
            # gate, so the PE stays throttled at 1.2 GHz. Keep the clock at
            # 2.4 GHz with real bf16 matmuls into a scratch PSUM bank: a
            # startup burst to trigger the un-throttle, then periodic
            # refreshers inside the loop.
            bf16 = mybir.dt.bfloat16
            wsrc = consts.tile([K1, NCHUNK], bf16)
            nc.gpsimd.memset(wsrc[:], 0.0)
            warm_ps = wps.tile([32, NCHUNK], f32)

            def warmer(n=1):
                for _ in range(n):
                    nc.tensor.matmul(warm_ps[:], wsrc[:, 0:32], wsrc[:])

            warmer(20)

            for i in range(BPC // PAIR):
                b0 = i * PAIR
                t1 = load1.tile([K1, PAIR, C], f32r, tag="t1")
                t2f = load2.tile([P2 + K2, PAIR, C], f32r, tag="t2")
                t2 = t2f[P2 : P2 + K2]
                nc.sync.dma_start(
                    t1[:], y_d[b0 : b0 + PAIR, 0:K1, :].rearrange("b n c -> n b c")
                )
                nc.scalar.dma_start(
                    t2, y_d[b0 : b0 + PAIR, K1:T, :].rearrange("b n c -> n b c")
                )
                osb = store.tile([HORIZON, PAIR, C], f32, tag="osb")
                for bb in range(PAIR):
                    for j in range(2):
                        ps = psum.tile([HORIZON, NCHUNK], f32, tag="ps")
                        rhs1 = t1[:, bb, j * NCHUNK : (j + 1) * NCHUNK]
                        rhs2 = t2[:, bb, j * NCHUNK : (j + 1) * NCHUNK]
                        nc.tensor.matmul(
                            ps[:], a1[:], rhs1, start=True, stop=False,
                        )
                        nc.tensor.matmul(
                            ps[:], a2, rhs2, start=False, stop=True,
                        )
                        nc.vector.tensor_scalar_add(
                            osb[:, bb, j * NCHUNK : (j + 1) * NCHUNK], ps[:], bias[:]
                        )
                    warmer()
                nc.gpsimd.dma_start(
                    out_d[b0 : b0 + PAIR, :, :].rearrange("b h c -> h b c"), osb[:]
                )

    nc.finalize()
    return nc, run_bass_kernel_spmd


def kernel(x: np.ndarray, y: np.ndarray, w: np.ndarray, b: np.ndarray) -> np.ndarray:
    global _RUNNER
    if _RUNNER is None:
        _RUNNER = _build()
    nc, run_spmd = _RUNNER

    A, bias_vec = _coeffs(np.asarray(w), np.asarray(b))
    At = _round_fp32r(np.ascontiguousarray(A.T))    # [168, 24]
    a1 = np.ascontiguousarray(At[:K1])
    a2 = np.ascontiguousarray(At[K1:])
    bias = np.ascontiguousarray(bias_vec[:, None])

    y = _round_fp32r(np.asarray(y, dtype=np.float32))
    in_maps = []
    for c in range(N_CORES):
        in_maps.append(
            {
                "y": y[c * BPC : (c + 1) * BPC],
                "a1": a1,
                "a2": a2,
                "bias": bias,
            }
        )
    res = run_spmd(nc, in_maps, core_ids=list(range(N_CORES)))
    return np.concatenate([r["out"] for r in res.results], axis=0)
